# revision 25
# baseline (speedup 1.0000x reference)
"""Trainium2 Bass kernel for the soft-LUT cellular-ASIC module (fast path).

Math per layer:  state'[b,hw] = clip( sum_p tw[l,p,hw] * prod_m f(c_m, bit_m(p)) )
where c_m[b,hw] = state[b,(h+i)%32,(w+j-1)%32]  (m = i*3+j),  f(c,0)=1-c, f(c,1)=c,
tw = sigmoid(toggle_gates).  bit_m(p) = bit (8-m) of p (m=0 is the MSB).

Implementation: 9-level lerp tree ("soft-LUT contraction"), evaluated in f16
batched across all 16 position tiles at once.  Layout: partition p = ph*32+w
(ph = h%4, w), tile lane t = b*8+th (th = h//4).  The tree state for an engine
slice lives as A[:, q*nt + b*TH + th] (q = remaining LUT combos, th-minor), so
each level is 3 large tensor_tensor ops:
    d = A_hi - A_lo ; e = d * c_bcast ; A' = e + A_lo
with c_bcast a stride-0 broadcast AP over q (keeps the f16 2x DVE mode: the
cost model only requires the innermost AP dim packed).

Window gathers are done on-chip: h-rolls are quadrant-aligned partition copies
(+ th-shift pieces for ph wrap), w-rolls are stream_shuffle ops (within-32
partition permutation).  No DRAM round-trip between layers.

Engine split: DVE owns th 0..4 (10 of 16 lanes), Pool (gpsimd) owns th 5..7;
the two tree chains are fully independent per layer and only join at the
[128,16] state tile.  Act does the sigmoids (strided interleaved writes),
prefetched one layer ahead; toggle gates stream in as f16, one layer per DMA.

Sharding: data-parallel over batch B=16 across 8 cores (B_local=2, no comms).
"""

import numpy as np

import concourse.bass as bass
import concourse.bacc as bacc
import concourse.mybir as mybir
from concourse import tile
from concourse.bass_utils import run_bass_kernel_spmd

F32 = mybir.dt.float32
F16 = mybir.dt.float16
AF = mybir.ActivationFunctionType
OP = mybir.AluOpType

L = 4          # layers
NPOS = 512     # 2^9 LUT combos
HW = 1024      # 32*32 grid
BLOC = 2       # batch per core (16 / 8 cores)
NCORES = 8
THV = 5        # th lanes on DVE (th 0..4)
THP = 3        # th lanes on Pool (th 5..7)

MASK_M = [(w - 1) % 32 for w in range(32)]   # j=0: read w-1
MASK_P = [(w + 1) % 32 for w in range(32)]   # j=2: read w+1

# Window element contracted at each tree level.  Chosen so the multiplier for
# level s is available as late as possible is NOT needed: level 0 uses m=1
# (multiplier == state itself, no shuffle), r1-based elements sit mid-tree,
# r2-based ones last.  The host permutes the LUT q-axis to match (bit (8-s)
# of the layout index corresponds to window element LEVEL_M[s]).
LEVEL_M = [1, 0, 2, 4, 3, 5, 7, 6, 8]

_CACHE = {}


def _q_perm():
    """idx[q_layout] = original LUT combo p, per LEVEL_M bit order."""
    idx = np.zeros(NPOS, dtype=np.int64)
    for q in range(NPOS):
        p = 0
        for s in range(9):
            bit = (q >> (8 - s)) & 1
            p |= bit << (8 - LEVEL_M[s])
        idx[q] = p
    return idx


def _emit_rolled(nc, eng, dst, src):
    """dst = src rolled by +1 in h (PM layout [128, 16], t = b*8+th)."""
    # ph 0..2 rows: partition shift +32 (quadrant-aligned pieces)
    eng.tensor_copy(out=dst[0:32, :], in_=src[32:64, :])
    eng.tensor_copy(out=dst[32:64, :], in_=src[64:96, :])
    eng.tensor_copy(out=dst[64:96, :], in_=src[96:128, :])
    # ph=3 rows: h+1 lands in th+1 (with th 7 -> 0 wrap within the same b)
    dv = dst[96:128, :].rearrange("p (b th) -> p b th", b=2, th=8)
    sv = src[0:32, :].rearrange("p (b th) -> p b th", b=2, th=8)
    eng.tensor_copy(out=dv[:, :, 0:7], in_=sv[:, :, 1:8])
    eng.tensor_copy(out=dv[:, :, 7:8], in_=sv[:, :, 0:1])


SPLIT_LEVEL = 5  # levels >= SPLIT_LEVEL run merged on Pool


class _Slice:
    """One engine's tree slice (levels 0..SPLIT_LEVEL-1), emitted stepwise so
    the two engines' streams can be interleaved in dataflow order.  a0:
    [128, 512*TH] interleaved (q*TH + th)."""

    def __init__(self, eng, a0, cms, tho, TH, pool, tag):
        self.eng, self.a0, self.cms = eng, a0, cms
        self.tho, self.TH, self.pool, self.tag = tho, TH, pool, tag
        self.A = None

    def _cview(self, s, q):
        return (
            self.cms[s][:, :]
            .rearrange("p (b th) -> p b th", b=2, th=8)[
                :, :, self.tho : self.tho + self.TH
            ]
            .unsqueeze(1)
            .broadcast_to((128, q, 2, self.TH))
        )

    def sub0(self):
        Q, TH, a0 = 256, self.TH, self.a0
        self.d0 = self.pool.tile(
            [128, Q * TH], F16, tag=f"{self.tag}d0", name=f"{self.tag}d0t"
        )
        self.eng.tensor_sub(
            self.d0[:, :], a0[:, Q * TH : 2 * Q * TH], a0[:, 0 : Q * TH]
        )

    def level0(self):
        Q, TH, nt = 256, self.TH, 2 * self.TH
        d0v = (
            self.d0[:, :]
            .rearrange("p (q th) -> p q th", q=Q, th=TH)
            .unsqueeze(2)
            .broadcast_to((128, Q, 2, TH))
        )
        a0lo = (
            self.a0[:, 0 : Q * TH]
            .rearrange("p (q th) -> p q th", q=Q, th=TH)
            .unsqueeze(2)
            .broadcast_to((128, Q, 2, TH))
        )
        e0 = self.pool.tile(
            [128, Q * nt], F16, tag=f"{self.tag}e0", name=f"{self.tag}e0t"
        )
        e0v = e0[:, :].rearrange("p (q b th) -> p q b th", q=Q, b=2, th=TH)
        self.eng.tensor_tensor(out=e0v, in0=d0v, in1=self._cview(0, Q), op=OP.mult)
        A = self.pool.tile(
            [128, Q * nt], F16, tag=f"{self.tag}A1", name=f"{self.tag}A1t"
        )
        Av = A[:, :].rearrange("p (q b th) -> p q b th", q=Q, b=2, th=TH)
        self.eng.tensor_tensor(out=Av, in0=e0v, in1=a0lo, op=OP.add)
        self.A = A

    def level(self, s):
        TH, nt = self.TH, 2 * self.TH
        Qh = 256 >> s
        A = self.A
        d = self.pool.tile(
            [128, Qh * nt], F16, tag=f"{self.tag}d{s}", name=f"{self.tag}d{s}t"
        )
        self.eng.tensor_sub(
            d[:, :], A[:, Qh * nt : 2 * Qh * nt], A[:, 0 : Qh * nt]
        )
        dv = d[:, :].rearrange("p (q b th) -> p q b th", q=Qh, b=2, th=TH)
        e = self.pool.tile(
            [128, Qh * nt], F16, tag=f"{self.tag}e{s}", name=f"{self.tag}e{s}t"
        )
        ev = e[:, :].rearrange("p (q b th) -> p q b th", q=Qh, b=2, th=TH)
        self.eng.tensor_tensor(out=ev, in0=dv, in1=self._cview(s, Qh), op=OP.mult)
        alo = A[:, 0 : Qh * nt].rearrange(
            "p (q b th) -> p q b th", q=Qh, b=2, th=TH
        )
        A2 = self.pool.tile(
            [128, Qh * nt], F16, tag=f"{self.tag}A{s+1}", name=f"{self.tag}A{s+1}t"
        )
        A2v = A2[:, :].rearrange("p (q b th) -> p q b th", q=Qh, b=2, th=TH)
        self.eng.tensor_tensor(out=A2v, in0=ev, in1=alo, op=OP.add)
        self.A = A2


def _emit_wtail(nc, ucs, pool):
    """Build W16[q*16+t] = prod over the last 4 bits of f(c, bit) from the
    uc tiles ([128,32]: [0:16]=1-c, [16:32]=c).  Runs on Pool, entirely off
    the layer-critical path (multipliers are ready near the layer start)."""

    def ucv(s, nj):
        # (j, b, t) view of uc_s with j broadcast (nj values)
        return ucs[s][:, :].rearrange("p (b t) -> p b t", b=2, t=16).unsqueeze(
            1
        ).broadcast_to((128, nj, 2, 16))

    x1 = pool.tile([128, 64], F16, tag="wx1")
    x1v = x1[:, :].rearrange("p (a b t) -> p a b t", a=2, b=2, t=16)
    in0 = (
        ucs[SPLIT_LEVEL][:, :]
        .rearrange("p (b t) -> p b t", b=2, t=16)
        .unsqueeze(2)
        .broadcast_to((128, 2, 2, 16))
    )
    nc.gpsimd.tensor_tensor(out=x1v, in0=in0, in1=ucv(SPLIT_LEVEL + 1, 2), op=OP.mult)
    x2 = pool.tile([128, 128], F16, tag="wx2")
    x2v = x2[:, :].rearrange("p (a b t) -> p a b t", a=4, b=2, t=16)
    in0 = (
        x1[:, :]
        .rearrange("p (a t) -> p a t", a=4, t=16)
        .unsqueeze(2)
        .broadcast_to((128, 4, 2, 16))
    )
    nc.gpsimd.tensor_tensor(out=x2v, in0=in0, in1=ucv(SPLIT_LEVEL + 2, 4), op=OP.mult)
    w = pool.tile([128, 256], F16, tag="wt")
    wv = w[:, :].rearrange("p (a b t) -> p a b t", a=8, b=2, t=16)
    in0 = (
        x2[:, :]
        .rearrange("p (a t) -> p a t", a=8, t=16)
        .unsqueeze(2)
        .broadcast_to((128, 8, 2, 16))
    )
    nc.gpsimd.tensor_tensor(out=wv, in0=in0, in1=ucv(SPLIT_LEVEL + 3, 8), op=OP.mult)
    return w


def _emit_tail(nc, Av, Ap, w, st32, pool):
    """V = A5 * W16 per slice, then halving-add reduce over the 16 remaining
    combos into st32 [128,16] (f32 final add).  All on DVE: the state chain
    ends where the next layer's level-0 runs, so DVE never waits on Pool at
    layer boundaries."""
    Q = 256 >> (SPLIT_LEVEL - 1)
    v = pool.tile([128, Q * 16], F16, tag="vt")
    vv = v[:, :].rearrange("p (q b th) -> p q b th", q=Q, b=2, th=8)
    wvw = w[:, :].rearrange("p (q b th) -> p q b th", q=Q, b=2, th=8)
    nc.vector.tensor_tensor(
        out=vv[:, :, :, 0:THV],
        in0=Av[:, :].rearrange("p (q b th) -> p q b th", q=Q, b=2, th=THV),
        in1=wvw[:, :, :, 0:THV],
        op=OP.mult,
    )
    nc.vector.tensor_tensor(
        out=vv[:, :, :, THV:8],
        in0=Ap[:, :].rearrange("p (q b th) -> p q b th", q=Q, b=2, th=THP),
        in1=wvw[:, :, :, THV:8],
        op=OP.mult,
    )
    cur = v
    n = Q * 16
    while n > 32:
        nxt = pool.tile([128, n // 2], F16, tag=f"vr{n}", name=f"vr{n}t")
        nc.vector.tensor_add(nxt[:, :], cur[:, 0 : n // 2], cur[:, n // 2 : n])
        cur, n = nxt, n // 2
    nc.vector.tensor_add(st32[:, :], cur[:, 0:16], cur[:, 16:32])


def _build():
    nc = bacc.Bacc("TRN2", target_bir_lowering=False, debug=True)

    xpm = nc.declare_dram_parameter("xpm", [128, 16], F16, isOutput=False)
    # a0h: layer-0 LUT table pre-activated host-side (interleaved layout),
    # layers 1..3 stream in raw and are activated on-chip during the
    # previous layer's tree.
    a0h = nc.declare_dram_parameter("a0h", [128, 8 * NPOS], F16, isOutput=False)
    tgh = nc.declare_dram_parameter("tgh", [L, 128, 8 * NPOS], F16, isOutput=False)
    out = nc.declare_dram_parameter("out", [128, 16], F32, isOutput=True)

    with tile.TileContext(nc) as tc:
        with (
            tc.tile_pool(name="tg", bufs=2) as tgp,
            tc.tile_pool(name="a0", bufs=2) as a0p,
            tc.tile_pool(name="st", bufs=2) as stp,
            tc.tile_pool(name="cm", bufs=2) as cmp_,
            tc.tile_pool(name="trv", bufs=1) as trv,
            tc.tile_pool(name="trp", bufs=1) as trp,
        ):
            state = stp.tile([128, 16], F16, tag="state0")
            nc.sync.dma_start(out=state[:, :], in_=xpm[:, :])

            for l in range(L):
                # ---- prefetch + sigmoid (runs during previous layer's tree)
                a0v = a0p.tile([128, NPOS * THV], F16, tag="a0v")
                a0q = a0p.tile([128, NPOS * THP], F16, tag="a0q")
                if l == 0:
                    nc.sync.dma_start(out=a0v[:, :], in_=a0h[:, 0 : THV * NPOS])
                    nc.sync.dma_start(out=a0q[:, :], in_=a0h[:, THV * NPOS :])
                else:
                    tgt = tgp.tile([128, 8 * NPOS], F16, tag="tgt")
                    nc.sync.dma_start(
                        out=tgt[:, 0 : THV * NPOS], in_=tgh[l, :, 0 : THV * NPOS]
                    )
                    nc.sync.dma_start(
                        out=tgt[:, THV * NPOS :], in_=tgh[l, :, THV * NPOS :]
                    )
                    nc.scalar.activation(
                        a0v[:, :].rearrange("p (q th) -> p th q", q=NPOS, th=THV),
                        tgt[:, 0 : THV * NPOS].rearrange(
                            "p (th q) -> p th q", th=THV, q=NPOS
                        ),
                        AF.Sigmoid,
                    )
                    nc.scalar.activation(
                        a0q[:, :].rearrange("p (q th) -> p th q", q=NPOS, th=THP),
                        tgt[:, THV * NPOS :].rearrange(
                            "p (th q) -> p th q", th=THP, q=NPOS
                        ),
                        AF.Sigmoid,
                    )

                # ---- window multipliers from state (level s uses element
                # LEVEL_M[s]; the host permuted the LUT q-axis to match).
                # Levels >= SPLIT_LEVEL land in uc tiles ([0:16]=1-c,
                # [16:32]=c) feeding the product-weight tail.
                r1 = cmp_.tile([128, 16], F16, tag="r1")
                ucs = {}
                for s in range(SPLIT_LEVEL, 9):
                    ucs[s] = cmp_.tile([128, 32], F16, tag=f"uc{s}", name=f"uc{s}_t")
                r2 = ucs[6][:, 16:32]  # m=7 identity lives in uc6's c-half
                cms = [None] * 9
                cms[0] = state   # m=1: identity
                cms[3] = r1      # m=4
                for s in (1, 2, 4):
                    cms[s] = cmp_.tile([128, 16], F16, tag=f"cm{s}", name=f"cm{s}_t")
                # ---- interleaved emission, dataflow order (level-0
                # multiplier is the state itself: no shuffle on the
                # critical path; sub0 needs only a0 -> head start while the
                # previous layer's tail drains on Pool)
                sv = _Slice(nc.vector, a0v, cms, 0, THV, trv, "v")
                sq = _Slice(nc.gpsimd, a0q, cms, THV, THP, trp, "q")
                sv.sub0()
                sq.sub0()
                sv.level0()
                sq.level0()
                _emit_rolled(nc, nc.gpsimd, r1, state)
                nc.vector.stream_shuffle(cms[1][:, :], state[:, :], MASK_M)
                sv.level(1)
                sq.level(1)
                _emit_rolled(nc, nc.gpsimd, r2, r1)
                nc.vector.stream_shuffle(cms[2][:, :], state[:, :], MASK_P)
                sv.level(2)
                sq.level(2)
                nc.vector.stream_shuffle(cms[4][:, :], r1[:, :], MASK_M)
                nc.vector.stream_shuffle(ucs[5][:, 16:32], r1[:, :], MASK_P)
                nc.vector.stream_shuffle(ucs[7][:, 16:32], r2, MASK_M)
                nc.vector.stream_shuffle(ucs[8][:, 16:32], r2, MASK_P)
                for s in range(SPLIT_LEVEL, 9):
                    nc.gpsimd.tensor_scalar(
                        ucs[s][:, 0:16], ucs[s][:, 16:32], -1.0, 1.0, OP.mult, OP.add
                    )
                w = _emit_wtail(nc, ucs, trp)
                sv.level(3)
                sq.level(3)
                sv.level(4)
                sq.level(4)
                st32 = stp.tile([128, 16], F32, tag="st32")
                _emit_tail(nc, sv.A, sq.A, w, st32, trp)
                if l < L - 1:
                    newstate = stp.tile([128, 16], F16, tag="state")
                    nc.vector.tensor_scalar(
                        newstate[:, :], st32[:, :], 0.0, 1.0, OP.max, OP.min
                    )
                    state = newstate

            outsb = stp.tile([128, 16], F32, tag="outsb")
            nc.gpsimd.tensor_scalar(
                outsb[:, :], st32[:, :], 0.0, 1.0, OP.max, OP.min
            )
            nc.sync.dma_start(out=out[:, :], in_=outsb[:, :])

    nc.finalize()
    return nc


def _host_inputs(x, tg):
    """x: [16,32,32] f32; tg: [4,512,32,32] f32 -> per-core xpm + shared
    tgh/a0h.  tgh[l, p, th*512+q]; a0h[p, :2560] = sig(l0)[q*5+th (th<5)],
    a0h[p, 2560:] = sig(l0)[q*3+(th-5)]."""
    tgq = tg.reshape(L, NPOS, 8, 4, 32).transpose(0, 3, 4, 2, 1)  # l, ph, w, th, q
    tgq = tgq[..., _q_perm()]  # bit-order permutation (level s <-> LEVEL_M[s])
    tgh = np.ascontiguousarray(tgq.reshape(L, 128, 8 * NPOS)).astype(np.float16)
    sig0 = 1.0 / (1.0 + np.exp(-tgq[0].reshape(128, 8, NPOS).astype(np.float32)))
    sig0 = sig0.astype(np.float16)  # [p, th, q]
    a0h = np.concatenate(
        [
            sig0[:, 0:THV, :].transpose(0, 2, 1).reshape(128, NPOS * THV),
            sig0[:, THV:8, :].transpose(0, 2, 1).reshape(128, NPOS * THP),
        ],
        axis=1,
    )
    a0h = np.ascontiguousarray(a0h)
    xpms = []
    for c in range(NCORES):
        xc = x[BLOC * c : BLOC * (c + 1)].reshape(BLOC, 8, 4, 32)
        xpms.append(
            np.ascontiguousarray(xc.transpose(2, 3, 0, 1).reshape(128, 16)).astype(
                np.float16
            )
        )
    return xpms, tgh, a0h


def _unpack_out(pm):
    """pm: [128, 16] f32 -> [2, 32, 32]."""
    return np.ascontiguousarray(
        pm.reshape(4, 32, BLOC, 8).transpose(2, 3, 0, 1).reshape(BLOC, 32, 32)
    )


def _run(x, toggle_gates, trace=False):
    if "nc" not in _CACHE:
        _CACHE["nc"] = _build()
    nc = _CACHE["nc"]

    x = np.asarray(x, dtype=np.float32)
    tg = np.asarray(toggle_gates, dtype=np.float32)
    xpms, tgh, a0h = _host_inputs(x, tg)
    in_maps = [{"xpm": xpms[c], "tgh": tgh, "a0h": a0h} for c in range(NCORES)]

    res = run_bass_kernel_spmd(nc, in_maps, core_ids=list(range(NCORES)), trace=trace)
    outs = []
    for c in range(NCORES):
        pm = np.asarray(res.results[c]["out"])
        outs.append(_unpack_out(pm))
    full = np.concatenate(outs, axis=0)
    return full, res


def kernel(x, toggle_gates):
    full, _ = _run(x, toggle_gates, trace=False)
    return full


# revision 26
# speedup vs baseline: 1.0219x; 1.0219x over previous
"""Trainium2 Bass kernel for the soft-LUT cellular-ASIC module (fast path).

Math per layer:  state'[b,hw] = clip( sum_p tw[l,p,hw] * prod_m f(c_m, bit_m(p)) )
where c_m[b,hw] = state[b,(h+i)%32,(w+j-1)%32]  (m = i*3+j),  f(c,0)=1-c, f(c,1)=c,
tw = sigmoid(toggle_gates).  bit_m(p) = bit (8-m) of p (m=0 is the MSB).

Implementation: 9-level lerp tree ("soft-LUT contraction"), evaluated in f16
batched across all 16 position tiles at once.  Layout: partition p = ph*32+w
(ph = h%4, w), tile lane t = b*8+th (th = h//4).  The tree state for an engine
slice lives as A[:, q*nt + b*TH + th] (q = remaining LUT combos, th-minor), so
each level is 3 large tensor_tensor ops:
    d = A_hi - A_lo ; e = d * c_bcast ; A' = e + A_lo
with c_bcast a stride-0 broadcast AP over q (keeps the f16 2x DVE mode: the
cost model only requires the innermost AP dim packed).

Window gathers are done on-chip: h-rolls are quadrant-aligned partition copies
(+ th-shift pieces for ph wrap), w-rolls are stream_shuffle ops (within-32
partition permutation).  No DRAM round-trip between layers.

Engine split: DVE owns th 0..4 (10 of 16 lanes), Pool (gpsimd) owns th 5..7;
the two tree chains are fully independent per layer and only join at the
[128,16] state tile.  Act does the sigmoids (strided interleaved writes),
prefetched one layer ahead; toggle gates stream in as f16, one layer per DMA.

Sharding: data-parallel over batch B=16 across 8 cores (B_local=2, no comms).
"""

import numpy as np

import concourse.bass as bass
import concourse.bacc as bacc
import concourse.mybir as mybir
from concourse import tile
from concourse.bass_utils import run_bass_kernel_spmd

F32 = mybir.dt.float32
F16 = mybir.dt.float16
AF = mybir.ActivationFunctionType
OP = mybir.AluOpType

L = 4          # layers
NPOS = 512     # 2^9 LUT combos
HW = 1024      # 32*32 grid
BLOC = 2       # batch per core (16 / 8 cores)
NCORES = 8
THV = 5        # th lanes on DVE (th 0..4)
THP = 3        # th lanes on Pool (th 5..7)

MASK_M = [(w - 1) % 32 for w in range(32)]   # j=0: read w-1
MASK_P = [(w + 1) % 32 for w in range(32)]   # j=2: read w+1

# Window element contracted at each tree level.  Chosen so the multiplier for
# level s is available as late as possible is NOT needed: level 0 uses m=1
# (multiplier == state itself, no shuffle), r1-based elements sit mid-tree,
# r2-based ones last.  The host permutes the LUT q-axis to match (bit (8-s)
# of the layout index corresponds to window element LEVEL_M[s]).
LEVEL_M = [1, 0, 2, 4, 3, 5, 7, 6, 8]

_CACHE = {}


def _q_perm():
    """idx[q_layout] = original LUT combo p, per LEVEL_M bit order."""
    idx = np.zeros(NPOS, dtype=np.int64)
    for q in range(NPOS):
        p = 0
        for s in range(9):
            bit = (q >> (8 - s)) & 1
            p |= bit << (8 - LEVEL_M[s])
        idx[q] = p
    return idx


def _emit_rolled(nc, eng, dst, src):
    """dst = src rolled by +1 in h (PM layout [128, 16], t = b*8+th)."""
    # ph 0..2 rows: partition shift +32 (quadrant-aligned pieces)
    eng.tensor_copy(out=dst[0:32, :], in_=src[32:64, :])
    eng.tensor_copy(out=dst[32:64, :], in_=src[64:96, :])
    eng.tensor_copy(out=dst[64:96, :], in_=src[96:128, :])
    # ph=3 rows: h+1 lands in th+1 (with th 7 -> 0 wrap within the same b)
    dv = dst[96:128, :].rearrange("p (b th) -> p b th", b=2, th=8)
    sv = src[0:32, :].rearrange("p (b th) -> p b th", b=2, th=8)
    eng.tensor_copy(out=dv[:, :, 0:7], in_=sv[:, :, 1:8])
    eng.tensor_copy(out=dv[:, :, 7:8], in_=sv[:, :, 0:1])


SPLIT_LEVEL = 5  # levels >= SPLIT_LEVEL run merged on Pool


class _Slice:
    """One engine's tree slice (levels 0..SPLIT_LEVEL-1), emitted stepwise so
    the two engines' streams can be interleaved in dataflow order.  a0:
    [128, 512*TH] interleaved (q*TH + th)."""

    def __init__(self, eng, a0, cms, tho, TH, pool, tag):
        self.eng, self.a0, self.cms = eng, a0, cms
        self.tho, self.TH, self.pool, self.tag = tho, TH, pool, tag
        self.A = None

    def _cview(self, s, q):
        return (
            self.cms[s][:, :]
            .rearrange("p (b th) -> p b th", b=2, th=8)[
                :, :, self.tho : self.tho + self.TH
            ]
            .unsqueeze(1)
            .broadcast_to((128, q, 2, self.TH))
        )

    def sub0(self):
        Q, TH, a0 = 256, self.TH, self.a0
        self.d0 = self.pool.tile(
            [128, Q * TH], F16, tag=f"{self.tag}d0", name=f"{self.tag}d0t"
        )
        self.eng.tensor_sub(
            self.d0[:, :], a0[:, Q * TH : 2 * Q * TH], a0[:, 0 : Q * TH]
        )

    def level0(self):
        Q, TH, nt = 256, self.TH, 2 * self.TH
        d0v = (
            self.d0[:, :]
            .rearrange("p (q th) -> p q th", q=Q, th=TH)
            .unsqueeze(2)
            .broadcast_to((128, Q, 2, TH))
        )
        a0lo = (
            self.a0[:, 0 : Q * TH]
            .rearrange("p (q th) -> p q th", q=Q, th=TH)
            .unsqueeze(2)
            .broadcast_to((128, Q, 2, TH))
        )
        e0 = self.pool.tile(
            [128, Q * nt], F16, tag=f"{self.tag}e0", name=f"{self.tag}e0t"
        )
        e0v = e0[:, :].rearrange("p (q b th) -> p q b th", q=Q, b=2, th=TH)
        self.eng.tensor_tensor(out=e0v, in0=d0v, in1=self._cview(0, Q), op=OP.mult)
        A = self.pool.tile(
            [128, Q * nt], F16, tag=f"{self.tag}A1", name=f"{self.tag}A1t"
        )
        Av = A[:, :].rearrange("p (q b th) -> p q b th", q=Q, b=2, th=TH)
        self.eng.tensor_tensor(out=Av, in0=e0v, in1=a0lo, op=OP.add)
        self.A = A

    def level(self, s):
        TH, nt = self.TH, 2 * self.TH
        Qh = 256 >> s
        A = self.A
        d = self.pool.tile(
            [128, Qh * nt], F16, tag=f"{self.tag}d{s}", name=f"{self.tag}d{s}t"
        )
        self.eng.tensor_sub(
            d[:, :], A[:, Qh * nt : 2 * Qh * nt], A[:, 0 : Qh * nt]
        )
        dv = d[:, :].rearrange("p (q b th) -> p q b th", q=Qh, b=2, th=TH)
        e = self.pool.tile(
            [128, Qh * nt], F16, tag=f"{self.tag}e{s}", name=f"{self.tag}e{s}t"
        )
        ev = e[:, :].rearrange("p (q b th) -> p q b th", q=Qh, b=2, th=TH)
        self.eng.tensor_tensor(out=ev, in0=dv, in1=self._cview(s, Qh), op=OP.mult)
        alo = A[:, 0 : Qh * nt].rearrange(
            "p (q b th) -> p q b th", q=Qh, b=2, th=TH
        )
        A2 = self.pool.tile(
            [128, Qh * nt], F16, tag=f"{self.tag}A{s+1}", name=f"{self.tag}A{s+1}t"
        )
        A2v = A2[:, :].rearrange("p (q b th) -> p q b th", q=Qh, b=2, th=TH)
        self.eng.tensor_tensor(out=A2v, in0=ev, in1=alo, op=OP.add)
        self.A = A2


def _emit_wtail(nc, ucs, pool):
    """Build W16[q*16+t] = prod over the last 4 bits of f(c, bit) from the
    uc tiles ([128,32]: [0:16]=1-c, [16:32]=c).  Runs on Pool, entirely off
    the layer-critical path (multipliers are ready near the layer start)."""

    def ucv(s, nj):
        # (j, b, t) view of uc_s with j broadcast (nj values)
        return ucs[s][:, :].rearrange("p (b t) -> p b t", b=2, t=16).unsqueeze(
            1
        ).broadcast_to((128, nj, 2, 16))

    x1 = pool.tile([128, 64], F16, tag="wx1")
    x1v = x1[:, :].rearrange("p (a b t) -> p a b t", a=2, b=2, t=16)
    in0 = (
        ucs[SPLIT_LEVEL][:, :]
        .rearrange("p (b t) -> p b t", b=2, t=16)
        .unsqueeze(2)
        .broadcast_to((128, 2, 2, 16))
    )
    nc.gpsimd.tensor_tensor(out=x1v, in0=in0, in1=ucv(SPLIT_LEVEL + 1, 2), op=OP.mult)
    x2 = pool.tile([128, 128], F16, tag="wx2")
    x2v = x2[:, :].rearrange("p (a b t) -> p a b t", a=4, b=2, t=16)
    in0 = (
        x1[:, :]
        .rearrange("p (a t) -> p a t", a=4, t=16)
        .unsqueeze(2)
        .broadcast_to((128, 4, 2, 16))
    )
    nc.gpsimd.tensor_tensor(out=x2v, in0=in0, in1=ucv(SPLIT_LEVEL + 2, 4), op=OP.mult)
    w = pool.tile([128, 256], F16, tag="wt")
    wv = w[:, :].rearrange("p (a b t) -> p a b t", a=8, b=2, t=16)
    in0 = (
        x2[:, :]
        .rearrange("p (a t) -> p a t", a=8, t=16)
        .unsqueeze(2)
        .broadcast_to((128, 8, 2, 16))
    )
    nc.gpsimd.tensor_tensor(out=wv, in0=in0, in1=ucv(SPLIT_LEVEL + 3, 8), op=OP.mult)
    return w


def _emit_tail(nc, Av, Ap, w, st32, pool):
    """V = A5 * W16 per slice, then halving-add reduce over the 16 remaining
    combos into st32 [128,16] (f32 final add).  All on DVE: the state chain
    ends where the next layer's level-0 runs, so DVE never waits on Pool at
    layer boundaries."""
    Q = 256 >> (SPLIT_LEVEL - 1)
    v = pool.tile([128, Q * 16], F16, tag="vt")
    vv = v[:, :].rearrange("p (q b th) -> p q b th", q=Q, b=2, th=8)
    wvw = w[:, :].rearrange("p (q b th) -> p q b th", q=Q, b=2, th=8)
    nc.gpsimd.tensor_tensor(
        out=vv[:, :, :, 0:THV],
        in0=Av[:, :].rearrange("p (q b th) -> p q b th", q=Q, b=2, th=THV),
        in1=wvw[:, :, :, 0:THV],
        op=OP.mult,
    )
    nc.gpsimd.tensor_tensor(
        out=vv[:, :, :, THV:8],
        in0=Ap[:, :].rearrange("p (q b th) -> p q b th", q=Q, b=2, th=THP),
        in1=wvw[:, :, :, THV:8],
        op=OP.mult,
    )
    cur = v
    n = Q * 16
    while n > 32:
        nxt = pool.tile([128, n // 2], F16, tag=f"vr{n}", name=f"vr{n}t")
        nc.gpsimd.tensor_add(nxt[:, :], cur[:, 0 : n // 2], cur[:, n // 2 : n])
        cur, n = nxt, n // 2
    nc.gpsimd.tensor_add(st32[:, :], cur[:, 0:16], cur[:, 16:32])


def _build():
    nc = bacc.Bacc("TRN2", target_bir_lowering=False, debug=True)

    xpm = nc.declare_dram_parameter("xpm", [128, 16], F16, isOutput=False)
    # a0h: layer-0 LUT table pre-activated host-side (interleaved layout),
    # layers 1..3 stream in raw and are activated on-chip during the
    # previous layer's tree.
    a0h = nc.declare_dram_parameter("a0h", [128, 8 * NPOS], F16, isOutput=False)
    tgh = nc.declare_dram_parameter("tgh", [L, 128, 8 * NPOS], F16, isOutput=False)
    out = nc.declare_dram_parameter("out", [128, 16], F32, isOutput=True)

    with tile.TileContext(nc) as tc:
        with (
            tc.tile_pool(name="tg", bufs=2) as tgp,
            tc.tile_pool(name="a0", bufs=2) as a0p,
            tc.tile_pool(name="st", bufs=2) as stp,
            tc.tile_pool(name="cm", bufs=2) as cmp_,
            tc.tile_pool(name="trv", bufs=1) as trv,
            tc.tile_pool(name="trp", bufs=1) as trp,
        ):
            state = stp.tile([128, 16], F16, tag="state0")
            nc.sync.dma_start(out=state[:, :], in_=xpm[:, :])

            for l in range(L):
                # ---- prefetch + sigmoid (runs during previous layer's tree)
                a0v = a0p.tile([128, NPOS * THV], F16, tag="a0v")
                a0q = a0p.tile([128, NPOS * THP], F16, tag="a0q")
                if l == 0:
                    nc.sync.dma_start(out=a0v[:, :], in_=a0h[:, 0 : THV * NPOS])
                    nc.sync.dma_start(out=a0q[:, :], in_=a0h[:, THV * NPOS :])
                else:
                    tgt = tgp.tile([128, 8 * NPOS], F16, tag="tgt")
                    nc.sync.dma_start(
                        out=tgt[:, 0 : THV * NPOS], in_=tgh[l, :, 0 : THV * NPOS]
                    )
                    nc.sync.dma_start(
                        out=tgt[:, THV * NPOS :], in_=tgh[l, :, THV * NPOS :]
                    )
                    nc.scalar.activation(
                        a0v[:, :].rearrange("p (q th) -> p th q", q=NPOS, th=THV),
                        tgt[:, 0 : THV * NPOS].rearrange(
                            "p (th q) -> p th q", th=THV, q=NPOS
                        ),
                        AF.Sigmoid,
                    )
                    nc.scalar.activation(
                        a0q[:, :].rearrange("p (q th) -> p th q", q=NPOS, th=THP),
                        tgt[:, THV * NPOS :].rearrange(
                            "p (th q) -> p th q", th=THP, q=NPOS
                        ),
                        AF.Sigmoid,
                    )

                # ---- window multipliers from state (level s uses element
                # LEVEL_M[s]; the host permuted the LUT q-axis to match).
                # Levels >= SPLIT_LEVEL land in uc tiles ([0:16]=1-c,
                # [16:32]=c) feeding the product-weight tail.
                r1 = cmp_.tile([128, 16], F16, tag="r1")
                ucs = {}
                for s in range(SPLIT_LEVEL, 9):
                    ucs[s] = cmp_.tile([128, 32], F16, tag=f"uc{s}", name=f"uc{s}_t")
                r2 = ucs[6][:, 16:32]  # m=7 identity lives in uc6's c-half
                cms = [None] * 9
                cms[0] = state   # m=1: identity
                cms[3] = r1      # m=4
                for s in (1, 2, 4):
                    cms[s] = cmp_.tile([128, 16], F16, tag=f"cm{s}", name=f"cm{s}_t")
                # ---- interleaved emission, dataflow order (level-0
                # multiplier is the state itself: no shuffle on the
                # critical path; sub0 needs only a0 -> head start while the
                # previous layer's tail drains on Pool)
                sv = _Slice(nc.vector, a0v, cms, 0, THV, trv, "v")
                sq = _Slice(nc.gpsimd, a0q, cms, THV, THP, trp, "q")
                sv.sub0()
                sq.sub0()
                sv.level0()
                sq.level0()
                _emit_rolled(nc, nc.gpsimd, r1, state)
                nc.vector.stream_shuffle(cms[1][:, :], state[:, :], MASK_M)
                sv.level(1)
                sq.level(1)
                _emit_rolled(nc, nc.gpsimd, r2, r1)
                nc.vector.stream_shuffle(cms[2][:, :], state[:, :], MASK_P)
                sv.level(2)
                sq.level(2)
                nc.vector.stream_shuffle(cms[4][:, :], r1[:, :], MASK_M)
                nc.vector.stream_shuffle(ucs[5][:, 16:32], r1[:, :], MASK_P)
                nc.vector.stream_shuffle(ucs[7][:, 16:32], r2, MASK_M)
                nc.vector.stream_shuffle(ucs[8][:, 16:32], r2, MASK_P)
                for s in range(SPLIT_LEVEL, 9):
                    nc.gpsimd.tensor_scalar(
                        ucs[s][:, 0:16], ucs[s][:, 16:32], -1.0, 1.0, OP.mult, OP.add
                    )
                w = _emit_wtail(nc, ucs, trp)
                sv.level(3)
                sq.level(3)
                sv.level(4)
                sq.level(4)
                st32 = stp.tile([128, 16], F32, tag="st32")
                _emit_tail(nc, sv.A, sq.A, w, st32, trp)
                if l < L - 1:
                    newstate = stp.tile([128, 16], F16, tag="state")
                    nc.gpsimd.tensor_scalar(
                        newstate[:, :], st32[:, :], 0.0, 1.0, OP.max, OP.min
                    )
                    state = newstate

            outsb = stp.tile([128, 16], F32, tag="outsb")
            nc.gpsimd.tensor_scalar(
                outsb[:, :], st32[:, :], 0.0, 1.0, OP.max, OP.min
            )
            nc.sync.dma_start(out=out[:, :], in_=outsb[:, :])

    nc.finalize()
    return nc


def _host_inputs(x, tg):
    """x: [16,32,32] f32; tg: [4,512,32,32] f32 -> per-core xpm + shared
    tgh/a0h.  tgh[l, p, th*512+q]; a0h[p, :2560] = sig(l0)[q*5+th (th<5)],
    a0h[p, 2560:] = sig(l0)[q*3+(th-5)]."""
    tgq = tg.reshape(L, NPOS, 8, 4, 32).transpose(0, 3, 4, 2, 1)  # l, ph, w, th, q
    tgq = tgq[..., _q_perm()]  # bit-order permutation (level s <-> LEVEL_M[s])
    tgh = np.ascontiguousarray(tgq.reshape(L, 128, 8 * NPOS)).astype(np.float16)
    sig0 = 1.0 / (1.0 + np.exp(-tgq[0].reshape(128, 8, NPOS).astype(np.float32)))
    sig0 = sig0.astype(np.float16)  # [p, th, q]
    a0h = np.concatenate(
        [
            sig0[:, 0:THV, :].transpose(0, 2, 1).reshape(128, NPOS * THV),
            sig0[:, THV:8, :].transpose(0, 2, 1).reshape(128, NPOS * THP),
        ],
        axis=1,
    )
    a0h = np.ascontiguousarray(a0h)
    xpms = []
    for c in range(NCORES):
        xc = x[BLOC * c : BLOC * (c + 1)].reshape(BLOC, 8, 4, 32)
        xpms.append(
            np.ascontiguousarray(xc.transpose(2, 3, 0, 1).reshape(128, 16)).astype(
                np.float16
            )
        )
    return xpms, tgh, a0h


def _unpack_out(pm):
    """pm: [128, 16] f32 -> [2, 32, 32]."""
    return np.ascontiguousarray(
        pm.reshape(4, 32, BLOC, 8).transpose(2, 3, 0, 1).reshape(BLOC, 32, 32)
    )


def _run(x, toggle_gates, trace=False):
    if "nc" not in _CACHE:
        _CACHE["nc"] = _build()
    nc = _CACHE["nc"]

    x = np.asarray(x, dtype=np.float32)
    tg = np.asarray(toggle_gates, dtype=np.float32)
    xpms, tgh, a0h = _host_inputs(x, tg)
    in_maps = [{"xpm": xpms[c], "tgh": tgh, "a0h": a0h} for c in range(NCORES)]

    res = run_bass_kernel_spmd(nc, in_maps, core_ids=list(range(NCORES)), trace=trace)
    outs = []
    for c in range(NCORES):
        pm = np.asarray(res.results[c]["out"])
        outs.append(_unpack_out(pm))
    full = np.concatenate(outs, axis=0)
    return full, res


def kernel(x, toggle_gates):
    full, _ = _run(x, toggle_gates, trace=False)
    return full
